# revision 14
# baseline (speedup 1.0000x reference)
"""HalfKP-NNUE embedding-bag + MLP kernel for 8 Trainium2 NeuronCores.

Strategy (pure data parallel, batch sharded 8-way, params replicated):
  The embedding-bag sum0[b] = sum_k w1[0][idx0[b,k]] is computed as a
  count-matrix matmul: C[v,b] = #occurrences of v in row b's indices, then
  sum0 = T^T @ C on the tensor engine (the only unit with enough internal
  bandwidth -- the gathered rows never materialize).

  The count matrix is built exactly and duplicate-free:
    - host: per-row sort of the 30 indices (the reference sum is
      permutation-invariant), idx1 offset by +640 (table flattening), int16
    - device: run-length encode each row with a DVE prefix scan; only the
      last element of each equal-run scatters (value -> run length) via a
      GPSIMD local_scatter into C_T [128b, 1280v] (bf16, counts exact)
    - PE transposes C_T chunks and accumulates 20 [128x128]x[128x128]
      matmuls per tile against the SBUF-resident bf16 table
  Activations keep batch on the free dim throughout, so the 512->32->32->1
  MLP runs as plain matmuls with per-partition ACT bias+relu and no
  activation transposes.
"""

import numpy as np
import ml_dtypes

import concourse.bacc as bacc
import concourse.mybir as mybir
import concourse.tile as tile
import concourse.bass_utils as bass_utils
from concourse.bass_interp import get_hw_module
from concourse.masks import make_identity

N_CORES = 8
B = 65536
BC = B // N_CORES          # rows per core
P = 128                    # partitions / tile rows
K = 30                     # active features per table
K2 = 2 * K                 # both tables
V = 640
V2 = 2 * V                 # flattened vocab
H = 256
MH = 32
N_TILES = BC // P          # 64

F16 = np.float16

_CACHE = {}


def build_module(n_tiles=N_TILES):
    dt = mybir.dt
    nc = bacc.Bacc("TRN2", target_bir_lowering=False, debug=False,
                   num_devices=N_CORES)
    idx_d = nc.dram_tensor("idx", [P, n_tiles * K2], dt.int16,
                           kind="ExternalInput").ap()
    w1p_d = nc.dram_tensor("w1p", [P, 20 * P], dt.float16,
                           kind="ExternalInput").ap()
    fc2w_d = nc.dram_tensor("fc2w", [P, 4 * MH], dt.float16,
                            kind="ExternalInput").ap()
    fc3w_d = nc.dram_tensor("fc3w", [MH, MH], dt.float16,
                            kind="ExternalInput").ap()
    fc4w_d = nc.dram_tensor("fc4w", [MH, 1], dt.float16,
                            kind="ExternalInput").ap()
    fcb_d = nc.dram_tensor("fcb", [MH, 3], dt.float32,
                           kind="ExternalInput").ap()
    out_d = nc.dram_tensor("out", [1, n_tiles * P], dt.float32,
                           kind="ExternalOutput").ap()

    with tile.TileContext(nc) as tc:
        _body(nc, tc, n_tiles, idx_d, w1p_d, fc2w_d, fc3w_d, fc4w_d, fcb_d,
              out_d)
    nc.compile()
    return nc


def _body(nc, tc, n_tiles, idx_d, w1p_d, fc2w_d, fc3w_d, fc4w_d, fcb_d,
          out_d):
    dt = mybir.dt
    Relu = mybir.ActivationFunctionType.Relu
    Ident = mybir.ActivationFunctionType.Identity
    Alu = mybir.AluOpType

    from contextlib import ExitStack
    ctx = ExitStack()
    const = ctx.enter_context(tc.tile_pool(name="const", bufs=1))
    work = ctx.enter_context(tc.tile_pool(name="work", bufs=4))
    psum = ctx.enter_context(tc.tile_pool(name="psum", bufs=2, space="PSUM"))

    # persistent inputs
    idx_sb = const.tile([P, n_tiles * K2], dt.int16)
    nc.sync.dma_start(out=idx_sb[:], in_=idx_d[:])
    w1p = const.tile([P, 20 * P], dt.float16)
    nc.sync.dma_start(out=w1p[:], in_=w1p_d[:])
    fc2w = const.tile([P, 4 * MH], dt.float16)
    nc.sync.dma_start(out=fc2w[:], in_=fc2w_d[:])
    fc3w = const.tile([MH, MH], dt.float16)
    nc.sync.dma_start(out=fc3w[:], in_=fc3w_d[:])
    fc4w = const.tile([MH, 1], dt.float16)
    nc.sync.dma_start(out=fc4w[:], in_=fc4w_d[:])
    fcb = const.tile([MH, 3], dt.float32)
    nc.sync.dma_start(out=fcb[:], in_=fcb_d[:])

    ident = const.tile([P, P], dt.float16)
    make_identity(nc, ident[:])
    ones60 = const.tile([P, 4 * K2], dt.float16)
    nc.vector.memset(ones60[:], 1.0)
    neg1 = const.tile([P, 4 * K2], dt.int16)
    nc.vector.memset(neg1[:], -1)
    out_row = const.tile([1, n_tiles * P], dt.float32)

    assert n_tiles % 2 == 0
    NB = 2 * P     # 256 batch cols per pair
    G = 4 if n_tiles % 4 == 0 else 2   # tiles per run-length group
    KG = G * K2
    for grp in range(n_tiles // G):
        x = idx_sb[:, grp * KG:(grp + 1) * KG]  # [128, G*60], runs grouped
        # run-length chain for G tiles at once; the idx0/idx1 +640 offset
        # breaks runs at every 30-col segment boundary automatically, and
        # a tile's last col (>=640) never equals the next tile's first
        # (<640), so runs cannot span any segment or tile boundary.
        e = work.tile([P, KG], dt.float16, tag="e")
        nc.vector.memset(e[:, 0:1], 0.0)
        nc.vector.tensor_tensor(out=e[:, 1:KG], in0=x[:, 1:KG],
                                in1=x[:, 0:KG - 1], op=Alu.is_equal)
        rc = work.tile([P, KG], dt.float16, tag="rc")
        nc.vector.tensor_tensor_scan(out=rc[:], data0=e[:],
                                     data1=ones60[:, 0:KG],
                                     initial=0.0, op0=Alu.mult, op1=Alu.add)
        lm = work.tile([P, KG], dt.int16, tag="lm")
        nc.vector.tensor_tensor(out=lm[:, 0:KG - 1], in0=x[:, 0:KG - 1],
                                in1=x[:, 1:KG], op=Alu.not_equal)
        nc.vector.memset(lm[:, KG - 1:KG], 1)
        pos = work.tile([P, KG], dt.int16, tag="pos")
        nc.vector.select(out=pos[:], mask=lm[:], on_true=x, on_false=neg1[:])

        for half in range(G // 2):
            _pair_body(nc, work, psum, w1p, fc2w, fc3w, fc4w, fcb, ident,
                       out_row, rc, pos, half, grp * (G // 2) + half, dt)

    nc.sync.dma_start(out=out_d[:], in_=out_row[:])
    ctx.close()


def _pair_body(nc, work, psum, w1p, fc2w, fc3w, fc4w, fcb, ident, out_row,
               rc, pos, half, pr, dt):
    Relu = mybir.ActivationFunctionType.Relu
    Alu = mybir.AluOpType
    NB = 2 * P

    if True:
        cts = []
        for ti in range(2):
            s = (2 * half + ti) * K2
            ct = work.tile([P, V2], dt.float16, tag=f"ct{ti}")
            nc.gpsimd.local_scatter(ct[:], rc[:, s:s + K2],
                                    pos[:, s:s + K2],
                                    channels=P, num_elems=V2, num_idxs=K2)
            cts.append(ct)

        # transpose C_T -> C [v,b] chunks (PE); pair layout: chunk q holds
        # [128v, 256b] at csb cols [256q : 256q+256], tile ti in half ti.
        csb = work.tile([P, 10 * NB], dt.float16, tag="csb")
        csb_r = csb[:].rearrange("p (q t b) -> p q t b", t=2, b=P)
        for ti in range(2):
            ct = cts[ti]
            for g in range(3):
                ng = 4 if g < 2 else 2
                cps = psum.tile([P, 4 * P], dt.float16, tag="cps")
                for jj in range(ng):
                    j = 4 * g + jj
                    nc.tensor.transpose(out=cps[:, jj * P:(jj + 1) * P],
                                        in_=ct[:, j * P:(j + 1) * P],
                                        identity=ident[:])
                src = cps[:].rearrange("p (q b) -> p q b", b=P)[:, 0:ng, :]
                dst = csb_r[:, 4 * g:4 * g + ng, ti, :]
                if g < 2:
                    nc.vector.tensor_copy(out=dst, in_=src)
                else:
                    nc.scalar.copy(out=dst, in_=src)

        # main embedding matmuls: psum[h,b] += T[v,h]^T-block @ C[v,b]-block
        mps = psum.tile([P, 4 * NB], dt.float32, tag="mps")
        for t2 in range(2):
            for hh in range(2):
                m = t2 * 2 + hh
                for c in range(5):
                    blk = (t2 * 5 + c) * 2 + hh
                    q = t2 * 5 + c
                    nc.tensor.matmul(
                        out=mps[:, m * NB:(m + 1) * NB],
                        lhsT=w1p[:, blk * P:(blk + 1) * P],
                        rhs=csb[:, q * NB:(q + 1) * NB],
                        start=(c == 0), stop=(c == 4))

        hsb = work.tile([P, 4 * NB], dt.float16, tag="hsb")
        nc.scalar.activation(out=hsb[:], in_=mps[:], func=Relu)

        fps = psum.tile([MH, NB], dt.float32, tag="fps")
        for j in range(4):
            nc.tensor.matmul(out=fps[:],
                             lhsT=fc2w[:, j * MH:(j + 1) * MH],
                             rhs=hsb[:, j * NB:(j + 1) * NB],
                             start=(j == 0), stop=(j == 3))
        h2 = work.tile([MH, NB], dt.float16, tag="h2")
        nc.scalar.activation(out=h2[:], in_=fps[:], func=Relu,
                             bias=fcb[:, 0:1])
        fps3 = psum.tile([MH, NB], dt.float32, tag="fps")
        nc.tensor.matmul(out=fps3[:], lhsT=fc3w[:], rhs=h2[:],
                         start=True, stop=True)
        h3 = work.tile([MH, NB], dt.float16, tag="h3")
        nc.scalar.activation(out=h3[:], in_=fps3[:], func=Relu,
                             bias=fcb[:, 1:2])
        fps4 = psum.tile([MH, NB], dt.float32, tag="fps")
        nc.tensor.matmul(out=fps4[0:1, :], lhsT=fc4w[:], rhs=h3[:],
                         start=True, stop=True)
        nc.vector.tensor_scalar_add(out_row[:, pr * NB:(pr + 1) * NB],
                                    fps4[0:1, :], fcb[0:1, 2:3])


def _pack_weights(w1, fc2_w, fc2_b, fc3_w, fc3_b, fc4_w, fc4_b):
    w1 = np.asarray(w1, np.float32)
    blocks = []
    for t2 in range(2):
        for c in range(5):
            for hh in range(2):
                blocks.append(w1[t2, c * P:(c + 1) * P, hh * P:(hh + 1) * P])
    w1p = np.ascontiguousarray(np.concatenate(blocks, axis=1)).astype(F16)

    fc2wT = np.asarray(fc2_w, np.float32).T  # [512, 32]
    fc2wp = np.ascontiguousarray(np.concatenate(
        [fc2wT[j * P:(j + 1) * P] for j in range(4)], axis=1)).astype(F16)
    fc3wp = np.ascontiguousarray(np.asarray(fc3_w, np.float32).T).astype(F16)
    fc4wp = np.ascontiguousarray(np.asarray(fc4_w, np.float32).T).astype(F16)
    fcb = np.zeros((MH, 3), np.float32)
    fcb[:, 0] = np.asarray(fc2_b, np.float32)
    fcb[:, 1] = np.asarray(fc3_b, np.float32)
    fcb[0, 2] = np.asarray(fc4_b, np.float32).reshape(-1)[0]
    return w1p, fc2wp, fc3wp, fc4wp, fcb


def _pack_indices(idx0, idx1):
    i0 = np.sort(np.asarray(idx0, np.int64), axis=1)
    i1 = np.sort(np.asarray(idx1, np.int64), axis=1) + V
    return np.concatenate([i0, i1], axis=1).astype(np.int16)  # [B, 60]


def _run(trace, idx0, idx1, w1, fc2_w, fc2_b, fc3_w, fc3_b, fc4_w, fc4_b):
    idx01 = _pack_indices(idx0, idx1)
    w1p, fc2wp, fc3wp, fc4wp, fcb = _pack_weights(
        w1, fc2_w, fc2_b, fc3_w, fc3_b, fc4_w, fc4_b)

    if "nc" not in _CACHE:
        nc = build_module()
        nc.m = get_hw_module(nc.m)
        _CACHE["nc"] = nc
    nc = _CACHE["nc"]

    in_maps = []
    for c in range(N_CORES):
        sl = idx01[c * BC:(c + 1) * BC]
        tiles = np.ascontiguousarray(
            sl.reshape(N_TILES, P, K2).transpose(1, 0, 2).reshape(P, -1))
        in_maps.append({
            "idx": tiles, "w1p": w1p, "fc2w": fc2wp, "fc3w": fc3wp,
            "fc4w": fc4wp, "fcb": fcb,
        })

    return bass_utils.run_bass_kernel_spmd(
        nc, in_maps, core_ids=list(range(N_CORES)), trace=trace)


def run_traced(**inputs):
    return _run(True, **inputs)


def kernel(idx0, idx1, w1, fc2_w, fc2_b, fc3_w, fc3_b, fc4_w, fc4_b):
    res = _run(False, idx0=idx0, idx1=idx1, w1=w1, fc2_w=fc2_w, fc2_b=fc2_b,
               fc3_w=fc3_w, fc3_b=fc3_b, fc4_w=fc4_w, fc4_b=fc4_b)
    out = np.concatenate(
        [np.asarray(res.results[c]["out"], np.float32).reshape(-1)
         for c in range(N_CORES)])
    return out


# revision 18
# speedup vs baseline: 1.1129x; 1.1129x over previous
"""HalfKP-NNUE embedding-bag + MLP kernel for 8 Trainium2 NeuronCores.

Strategy (pure data parallel, batch sharded 8-way, params replicated):
  The embedding-bag sum0[b] = sum_k w1[0][idx0[b,k]] is computed as a
  count-matrix matmul: C[v,b] = #occurrences of v in row b's indices, then
  sum0 = T^T @ C on the tensor engine (the only unit with enough internal
  bandwidth -- the gathered rows never materialize).

  The count matrix is built exactly and duplicate-free:
    - host: per-row sort of the 30 indices (the reference sum is
      permutation-invariant), idx1 offset by +640 (table flattening), int16
    - device: run-length encode each row with a DVE prefix scan; only the
      last element of each equal-run scatters (value -> run length) via a
      GPSIMD local_scatter into C_T [128b, 1280v] (bf16, counts exact)
    - PE transposes C_T chunks and accumulates 20 [128x128]x[128x128]
      matmuls per tile against the SBUF-resident bf16 table
  Activations keep batch on the free dim throughout, so the 512->32->32->1
  MLP runs as plain matmuls with per-partition ACT bias+relu and no
  activation transposes.
"""

import numpy as np
import ml_dtypes

import concourse.bacc as bacc
import concourse.mybir as mybir
import concourse.tile as tile
import concourse.bass_utils as bass_utils
from concourse.bass_interp import get_hw_module
from concourse.masks import make_identity

N_CORES = 8
B = 65536
BC = B // N_CORES          # rows per core
P = 128                    # partitions / tile rows
K = 30                     # active features per table
K2 = 2 * K                 # both tables
V = 640
V2 = 2 * V                 # flattened vocab
H = 256
MH = 32
N_TILES = BC // P          # 64

F16 = np.float16

_CACHE = {}


def build_module(n_tiles=N_TILES):
    dt = mybir.dt
    nc = bacc.Bacc("TRN2", target_bir_lowering=False, debug=False,
                   num_devices=N_CORES)
    idx_d = nc.dram_tensor("idx", [P, n_tiles * K2], dt.int16,
                           kind="ExternalInput").ap()
    w1p_d = nc.dram_tensor("w1p", [P, 20 * P], dt.float16,
                           kind="ExternalInput").ap()
    fc2w_d = nc.dram_tensor("fc2w", [P, 4 * MH], dt.float16,
                            kind="ExternalInput").ap()
    fc3w_d = nc.dram_tensor("fc3w", [MH, MH], dt.float16,
                            kind="ExternalInput").ap()
    fc4w_d = nc.dram_tensor("fc4w", [MH, 1], dt.float16,
                            kind="ExternalInput").ap()
    fcb_d = nc.dram_tensor("fcb", [MH, 3], dt.float32,
                           kind="ExternalInput").ap()
    out_d = nc.dram_tensor("out", [1, n_tiles * P], dt.float32,
                           kind="ExternalOutput").ap()

    with tile.TileContext(nc) as tc:
        _body(nc, tc, n_tiles, idx_d, w1p_d, fc2w_d, fc3w_d, fc4w_d, fcb_d,
              out_d)
    nc.compile()
    return nc


def _body(nc, tc, n_tiles, idx_d, w1p_d, fc2w_d, fc3w_d, fc4w_d, fcb_d,
          out_d):
    dt = mybir.dt
    Relu = mybir.ActivationFunctionType.Relu
    Ident = mybir.ActivationFunctionType.Identity
    Alu = mybir.AluOpType

    from contextlib import ExitStack
    ctx = ExitStack()
    const = ctx.enter_context(tc.tile_pool(name="const", bufs=1))
    work = ctx.enter_context(tc.tile_pool(name="work", bufs=4))
    psum_c = ctx.enter_context(tc.tile_pool(name="psc", bufs=4, space="PSUM"))
    psum_m = ctx.enter_context(tc.tile_pool(name="psm", bufs=1, space="PSUM"))
    psum_f = ctx.enter_context(tc.tile_pool(name="psf", bufs=2, space="PSUM"))

    # persistent inputs
    idx_sb = const.tile([P, n_tiles * K2], dt.int16)
    nc.sync.dma_start(out=idx_sb[:], in_=idx_d[:])
    w1p = const.tile([P, 20 * P], dt.float16)
    nc.sync.dma_start(out=w1p[:], in_=w1p_d[:])
    fc2w = const.tile([P, 4 * MH], dt.float16)
    nc.sync.dma_start(out=fc2w[:], in_=fc2w_d[:])
    fc3w = const.tile([MH, MH], dt.float16)
    nc.sync.dma_start(out=fc3w[:], in_=fc3w_d[:])
    fc4w = const.tile([MH, 1], dt.float16)
    nc.sync.dma_start(out=fc4w[:], in_=fc4w_d[:])
    fcb = const.tile([MH, 3], dt.float32)
    nc.sync.dma_start(out=fcb[:], in_=fcb_d[:])

    ident = const.tile([P, P], dt.float16)
    make_identity(nc, ident[:])
    ones60 = const.tile([P, 4 * K2], dt.float16)
    nc.vector.memset(ones60[:], 1.0)
    neg1 = const.tile([P, 4 * K2], dt.int16)
    nc.vector.memset(neg1[:], -1)
    out_row = const.tile([1, n_tiles * P], dt.float32)

    assert n_tiles % 2 == 0
    NB = 2 * P     # 256 batch cols per pair
    G = 4 if n_tiles % 4 == 0 else 2   # tiles per run-length group
    KG = G * K2
    for grp in range(n_tiles // G):
        x = idx_sb[:, grp * KG:(grp + 1) * KG]  # [128, G*60], runs grouped
        # run-length chain for G tiles at once; the idx0/idx1 +640 offset
        # breaks runs at every 30-col segment boundary automatically, and
        # a tile's last col (>=640) never equals the next tile's first
        # (<640), so runs cannot span any segment or tile boundary.
        e = work.tile([P, KG], dt.float16, tag="e")
        nc.vector.memset(e[:, 0:1], 0.0)
        nc.vector.tensor_tensor(out=e[:, 1:KG], in0=x[:, 1:KG],
                                in1=x[:, 0:KG - 1], op=Alu.is_equal)
        rc = work.tile([P, KG], dt.float16, tag="rc")
        nc.vector.tensor_tensor_scan(out=rc[:], data0=e[:],
                                     data1=ones60[:, 0:KG],
                                     initial=0.0, op0=Alu.mult, op1=Alu.add)
        lm = work.tile([P, KG], dt.int16, tag="lm")
        nc.vector.tensor_tensor(out=lm[:, 0:KG - 1], in0=x[:, 0:KG - 1],
                                in1=x[:, 1:KG], op=Alu.not_equal)
        nc.vector.memset(lm[:, KG - 1:KG], 1)
        pos = work.tile([P, KG], dt.int16, tag="pos")
        nc.vector.select(out=pos[:], mask=lm[:], on_true=x,
                         on_false=neg1[:, 0:KG])

        # embedding sums for the group's tiles; relu lands in hsb_q with
        # hidden-chunk-major layout [p, chunk j(4), pair-half, 256b]
        NW = G * P
        hsb_q = work.tile([P, 4 * NW], dt.float16, tag="hsbq")
        for half in range(G // 2):
            _pair_body(nc, work, psum_c, psum_m, w1p, ident, hsb_q,
                       rc, pos, half, G, dt)

        # MLP once per group at N=512
        fps = psum_f.tile([MH, NW], dt.float32, tag="fps")
        for j in range(4):
            nc.tensor.matmul(out=fps[:],
                             lhsT=fc2w[:, j * MH:(j + 1) * MH],
                             rhs=hsb_q[:, j * NW:(j + 1) * NW],
                             start=(j == 0), stop=(j == 3))
        h2 = work.tile([MH, NW], dt.float16, tag="h2")
        nc.scalar.activation(out=h2[:], in_=fps[:], func=Relu,
                             bias=fcb[:, 0:1])
        fps3 = psum_f.tile([MH, NW], dt.float32, tag="fps")
        nc.tensor.matmul(out=fps3[:], lhsT=fc3w[:], rhs=h2[:],
                         start=True, stop=True)
        h3 = work.tile([MH, NW], dt.float16, tag="h3")
        nc.scalar.activation(out=h3[:], in_=fps3[:], func=Relu,
                             bias=fcb[:, 1:2])
        fps4 = psum_f.tile([MH, NW], dt.float32, tag="fps")
        nc.tensor.matmul(out=fps4[0:1, :], lhsT=fc4w[:], rhs=h3[:],
                         start=True, stop=True)
        nc.vector.tensor_scalar_add(out_row[:, grp * NW:(grp + 1) * NW],
                                    fps4[0:1, :], fcb[0:1, 2:3])

    nc.sync.dma_start(out=out_d[:], in_=out_row[:])
    ctx.close()


def _pair_body(nc, work, psum_c, psum_m, w1p, ident, hsb_q,
               rc, pos, half, G, dt):
    Relu = mybir.ActivationFunctionType.Relu
    NB = 2 * P

    cts = []
    for ti in range(2):
        s = (2 * half + ti) * K2
        ct = work.tile([P, V2], dt.float16, tag=f"ct{ti}")
        nc.gpsimd.local_scatter(ct[:], rc[:, s:s + K2],
                                pos[:, s:s + K2],
                                channels=P, num_elems=V2, num_idxs=K2)
        cts.append(ct)

    # transpose C_T -> C [v,b] chunks (PE); pair layout: chunk q holds
    # [128v, 256b] at csb cols [256q : 256q+256], tile ti in half ti.
    csb = work.tile([P, 10 * NB], dt.float16, tag="csb")
    csb_r = csb[:].rearrange("p (q t b) -> p q t b", t=2, b=P)
    for ti in range(2):
        ct = cts[ti]
        for g in range(3):
            ng = 4 if g < 2 else 2
            cps = psum_c.tile([P, 4 * P], dt.float16, tag="cps")
            for jj in range(ng):
                j = 4 * g + jj
                nc.tensor.transpose(out=cps[:, jj * P:(jj + 1) * P],
                                    in_=ct[:, j * P:(j + 1) * P],
                                    identity=ident[:])
            src = cps[:].rearrange("p (q b) -> p q b", b=P)[:, 0:ng, :]
            dst = csb_r[:, 4 * g:4 * g + ng, ti, :]
            if g < 2:
                nc.vector.tensor_copy(out=dst, in_=src)
            else:
                nc.scalar.copy(out=dst, in_=src)

    # main embedding matmuls: psum[h,b] += T[v,h]^T-block @ C[v,b]-block
    mps = psum_m.tile([P, 4 * NB], dt.float32, tag="mps")
    for t2 in range(2):
        for hh in range(2):
            m = t2 * 2 + hh
            for c in range(5):
                blk = (t2 * 5 + c) * 2 + hh
                q = t2 * 5 + c
                nc.tensor.matmul(
                    out=mps[:, m * NB:(m + 1) * NB],
                    lhsT=w1p[:, blk * P:(blk + 1) * P],
                    rhs=csb[:, q * NB:(q + 1) * NB],
                    start=(c == 0), stop=(c == 4))

    # relu into the group's hidden buffer, chunk-major strided slices
    hq = hsb_q[:].rearrange("p (j s b) -> p j s b", s=G // 2, b=NB)
    nc.scalar.activation(
        out=hq[:, :, half, :],
        in_=mps[:].rearrange("p (j b) -> p j b", b=NB),
        func=Relu)


def _pack_weights(w1, fc2_w, fc2_b, fc3_w, fc3_b, fc4_w, fc4_b):
    w1 = np.asarray(w1, np.float32)
    blocks = []
    for t2 in range(2):
        for c in range(5):
            for hh in range(2):
                blocks.append(w1[t2, c * P:(c + 1) * P, hh * P:(hh + 1) * P])
    w1p = np.ascontiguousarray(np.concatenate(blocks, axis=1)).astype(F16)

    fc2wT = np.asarray(fc2_w, np.float32).T  # [512, 32]
    fc2wp = np.ascontiguousarray(np.concatenate(
        [fc2wT[j * P:(j + 1) * P] for j in range(4)], axis=1)).astype(F16)
    fc3wp = np.ascontiguousarray(np.asarray(fc3_w, np.float32).T).astype(F16)
    fc4wp = np.ascontiguousarray(np.asarray(fc4_w, np.float32).T).astype(F16)
    fcb = np.zeros((MH, 3), np.float32)
    fcb[:, 0] = np.asarray(fc2_b, np.float32)
    fcb[:, 1] = np.asarray(fc3_b, np.float32)
    fcb[0, 2] = np.asarray(fc4_b, np.float32).reshape(-1)[0]
    return w1p, fc2wp, fc3wp, fc4wp, fcb


def _pack_indices(idx0, idx1):
    i0 = np.sort(np.asarray(idx0, np.int64), axis=1)
    i1 = np.sort(np.asarray(idx1, np.int64), axis=1) + V
    return np.concatenate([i0, i1], axis=1).astype(np.int16)  # [B, 60]


def _run(trace, idx0, idx1, w1, fc2_w, fc2_b, fc3_w, fc3_b, fc4_w, fc4_b):
    idx01 = _pack_indices(idx0, idx1)
    w1p, fc2wp, fc3wp, fc4wp, fcb = _pack_weights(
        w1, fc2_w, fc2_b, fc3_w, fc3_b, fc4_w, fc4_b)

    if "nc" not in _CACHE:
        nc = build_module()
        nc.m = get_hw_module(nc.m)
        _CACHE["nc"] = nc
    nc = _CACHE["nc"]

    in_maps = []
    for c in range(N_CORES):
        sl = idx01[c * BC:(c + 1) * BC]
        tiles = np.ascontiguousarray(
            sl.reshape(N_TILES, P, K2).transpose(1, 0, 2).reshape(P, -1))
        in_maps.append({
            "idx": tiles, "w1p": w1p, "fc2w": fc2wp, "fc3w": fc3wp,
            "fc4w": fc4wp, "fcb": fcb,
        })

    return bass_utils.run_bass_kernel_spmd(
        nc, in_maps, core_ids=list(range(N_CORES)), trace=trace)


def run_traced(**inputs):
    return _run(True, **inputs)


def kernel(idx0, idx1, w1, fc2_w, fc2_b, fc3_w, fc3_b, fc4_w, fc4_b):
    res = _run(False, idx0=idx0, idx1=idx1, w1=w1, fc2_w=fc2_w, fc2_b=fc2_b,
               fc3_w=fc3_w, fc3_b=fc3_b, fc4_w=fc4_w, fc4_b=fc4_b)
    out = np.concatenate(
        [np.asarray(res.results[c]["out"], np.float32).reshape(-1)
         for c in range(N_CORES)])
    return out
